# revision 2
# baseline (speedup 1.0000x reference)
"""DLRM-ResNet forward on 8 TRN2 cores — v2.

Data-parallel over batch (2048 samples/core); embedding table replicated
per core in fp8-e4m3 (values are uniform [0, 2^-11]; stored pre-scaled by
2^11 so they quantize well; the 2^-11 is folded into the top-MLP layer-0
scales). Per core:

  - 416 indirect gathers (one per slot x 128-sample subtile, 128B fp8 rows)
    on the Pool queue.
  - Per slot, one XBAR DMA-transpose of the gathered [128 samples, 1024B]
    block viewed as u16 (fp8 d-pairs), landing d-pairs on partitions:
    slot 2j -> partitions 0..63 of pair tile j, slot 2j+1 -> 64..127.
    Alternates between the two HWDGE queues (sync / scalar).
  - Embedding part of top-MLP layer 0 runs as fp8 DoubleRow matmuls
    (2 rows/cycle): lhsT [128, 2, 128] member-major, rhs [128, 2, 512]
    byte-strided view of the pair tile. Dense part accumulates into the
    same PSUM banks in f32r with weights pre-scaled 2^16 (= 2^11 table
    x 2^5 fp8-weight scale); one activation applies scale 2^-16 + bias.
  - Bottom/top MLP in f32r, feature-major; residual adds on DVE; ReLU+bias
    split between scalar ACT and DVE tensor_scalar to balance queues.
"""

import os
import numpy as np
import ml_dtypes

import concourse.bass as bass
import concourse.tile as tile
import concourse.mybir as mybir
from concourse.bass_utils import run_bass_kernel_spmd
from concourse.masks import make_identity

f8 = mybir.dt.float8e4
f16 = mybir.dt.float16
f32 = mybir.dt.float32
f32r = mybir.dt.float32r
u16 = mybir.dt.uint16
i32 = mybir.dt.int32

B, V, D = 16384, 4194304, 128
N_CORES = 8
BC = B // N_CORES          # 2048 samples per core
NSLOT = 26
NPAIR = 13
NJ = 16                    # 128-sample subtiles per core
NT = 4                     # batch tiles of 512
BOT = [13, 256, 256, 256]
TOP = [26 * D + 256, 256, 256, 256, 256, 1]

TSCALE = 2.0 ** 11         # table pre-scale
WSCALE = 2.0 ** 5          # tw0e fp8 pre-scale
PSCALE = TSCALE * WSCALE   # dense-path pre-scale / activation inverse scale

Relu = mybir.ActivationFunctionType.Relu
Ident = mybir.ActivationFunctionType.Identity


def _split_multi_waits(nc):
    """walrus accepts only ONE sync-wait per instruction; hoist extras onto
    same-engine EventSemaphore instructions immediately before."""
    n_new = 0
    for fn in nc.m.functions:
        for bb in fn.blocks:
            insts = list(bb.instructions)
            out = []
            changed = False
            for inst in insts:
                si = inst.sync_info
                waits = list(si.on_wait) if si is not None else []
                if len(waits) > 1:
                    changed = True
                    for w in waits[:-1]:
                        n_new += 1
                        out.append(mybir.InstEventSemaphore(
                            name=f"I-ws-{n_new}",
                            engine=inst.engine,
                            ins=[], outs=[],
                            sync_info=mybir.SyncInfo(on_wait=[w], on_update=[]),
                        ))
                    si.on_wait = [waits[-1]]
                out.append(inst)
            if changed:
                bb.instructions = out
    return n_new


def build_bass(bc=BC, v=V, split_waits=True):
    nc = bass.Bass("TRN2", target_bir_lowering=False, debug=False)
    nj = bc // 128
    nt = bc // 512

    xs = nc.dram_tensor("xs", [bc, 39], f32, kind="ExternalInput")
    table = nc.dram_tensor("table", [v, D], f8, kind="ExternalInput")
    # pair-layout fp8 weights: [p][pair, member, mhalf, m']
    tw0e8 = nc.dram_tensor("tw0e8", [128, NPAIR * 2 * 2 * 128], f8,
                           kind="ExternalInput")
    tw0h = nc.dram_tensor("tw0h", [256, 256], f32, kind="ExternalInput")
    ws = {}
    for i in range(3):
        ws[f"bw{i}"] = nc.dram_tensor(f"bw{i}", [BOT[i], BOT[i + 1]], f32,
                                      kind="ExternalInput")
        ws[f"bb{i}"] = nc.dram_tensor(f"bb{i}", [BOT[i + 1], 1], f32,
                                      kind="ExternalInput")
    for i in range(5):
        if i > 0:
            ws[f"tw{i}"] = nc.dram_tensor(f"tw{i}", [TOP[i], TOP[i + 1]], f32,
                                          kind="ExternalInput")
        ws[f"tb{i}"] = nc.dram_tensor(f"tb{i}", [TOP[i + 1], 1], f32,
                                      kind="ExternalInput")
    out = nc.dram_tensor("out", [bc, 1], f32, kind="ExternalOutput")

    with tile.TileContext(nc) as tc:
        with tc.tile_pool(name="const", bufs=1) as cpool, \
             tc.tile_pool(name="wpool", bufs=1) as wpool, \
             tc.tile_pool(name="epool", bufs=4) as epool, \
             tc.tile_pool(name="ztp", bufs=1) as ztpool, \
             tc.tile_pool(name="act", bufs=1) as apool, \
             tc.tile_pool(name="rlp", bufs=2) as rlpool, \
             tc.tile_pool(name="psb", bufs=1, space="PSUM") as psp:

            ident32 = cpool.tile([128, 128], f32)
            make_identity(nc, ident32[:])

            # ---- x load (sample s = 128*j + p -> partition p, block j) ----
            x_sb = cpool.tile([128, nj * 39], f32)
            nc.sync.dma_start(
                out=x_sb[:].rearrange("p (j d) -> p j d", d=39),
                in_=xs[:].rearrange("(j p) d -> p j d", p=128))

            # ---- indices: [p, (k, j)] ---------------------------------
            idx_f = cpool.tile([128, NSLOT * nj], f32)
            idx_i = cpool.tile([128, NSLOT * nj], i32)
            idx_m = cpool.tile([128, NSLOT * nj], i32)
            nc.vector.tensor_copy(
                out=idx_f[:].rearrange("p (k j) -> p k j", k=NSLOT),
                in_=x_sb[:].rearrange("p (j d) -> p j d", d=39)[:, :, 13:39]
                    .rearrange("p j k -> p k j"))
            nc.vector.tensor_copy(out=idx_i[:], in_=idx_f[:])
            nc.vector.tensor_scalar(
                out=idx_m[:], in0=idx_i[:], scalar1=v - 1, scalar2=None,
                op0=mybir.AluOpType.bitwise_and)

            # ---- weights ----------------------------------------------
            tw0e_sb = wpool.tile([128, NPAIR * 2 * 2 * 128], f8)
            nc.sync.dma_start(out=tw0e_sb[:], in_=tw0e8[:])
            tw0h_ld = wpool.tile([128, 2 * 256], f32, tag="tw0h_ld")
            nc.sync.dma_start(
                out=tw0h_ld[:].rearrange("p (k m) -> p k m", k=2),
                in_=tw0h[:].rearrange("(k p) m -> p k m", p=128))
            tw0h_sb = wpool.tile([128, 2 * 256], f32r)
            nc.vector.tensor_copy(out=tw0h_sb[:], in_=tw0h_ld[:].bitcast(f32r))
            mlp_w = {}
            for nm, fi, fo in [("bw1", 256, 256), ("bw2", 256, 256),
                               ("tw1", 256, 256), ("tw2", 256, 256),
                               ("tw3", 256, 256), ("tw4", 256, 1)]:
                tl = wpool.tile([128, (fi // 128) * fo], f32, tag=f"{nm}_ld")
                nc.sync.dma_start(
                    out=tl[:].rearrange("p (k m) -> p k m", k=fi // 128),
                    in_=ws[nm][:].rearrange("(k p) m -> p k m", p=128))
                t = wpool.tile([128, (fi // 128) * fo], f32r, tag=nm)
                nc.vector.tensor_copy(out=t[:], in_=tl[:].bitcast(f32r))
                mlp_w[nm] = t[:].rearrange("p (k m) -> p k m", k=fi // 128)
            bw0_ld = wpool.tile([13, 256], f32, tag="bw0_ld")
            nc.sync.dma_start(out=bw0_ld[:], in_=ws["bw0"][:])
            bw0_sb = wpool.tile([13, 256], f32r)
            nc.vector.tensor_copy(out=bw0_sb[:], in_=bw0_ld[:].bitcast(f32r))
            biases = {}
            for nm, fo in [("bb0", 256), ("bb1", 256), ("bb2", 256),
                           ("tb0", 256), ("tb1", 256), ("tb2", 256),
                           ("tb3", 256), ("tb4", 1)]:
                t = wpool.tile([min(fo, 128), (fo + 127) // 128], f32,
                               tag=f"b_{nm}")
                for m in range((fo + 127) // 128):
                    lo, hi = m * 128, min(fo, m * 128 + 128)
                    nc.sync.dma_start(out=t[: hi - lo, m:m + 1],
                                      in_=ws[nm][lo:hi, :])
                biases[nm] = t

            # ---- gathers + transposes ---------------------------------
            # e_pair: [p, (jj, a, d)] fp8 per slot-pair; one 2D XBAR per
            # (pair, jj): in [128 s, 128 u16] -> out [128 (a,u), 128 s],
            # so partitions land as q = 64*a + d//2, member = d parity.
            zts = [ztpool.tile([128, nj * 128], u16, tag=f"zt{j}",
                               name=f"zt{j}")
                   for j in range(NPAIR)]
            nxb = 0
            for j in range(NPAIR):
                e_t = epool.tile([128, 2 * nj * 128], f8, tag="ep")
                for jj in range(nj):
                    for a in range(2):
                        g = (2 * j + a) * nj + jj
                        nc.gpsimd.indirect_dma_start(
                            out=e_t[:, (jj * 2 + a) * 128:
                                    (jj * 2 + a + 1) * 128],
                            out_offset=None,
                            in_=table[:],
                            in_offset=bass.IndirectOffsetOnAxis(
                                ap=idx_m[:, g:g + 1], axis=0))
                    eng = nc.sync if nxb % 2 == 0 else nc.scalar
                    nxb += 1
                    eng.dma_start_transpose(
                        out=zts[j][:, jj * 128:(jj + 1) * 128],
                        in_=e_t[:].bitcast(u16)[:, jj * 128:(jj + 1) * 128])

            # ---- PSUM banks: tag ps{m}{t} -----------------------------
            def psum(m, t):
                return psp.tile([128, 512], f32, space="PSUM", tag=f"ps{m}{t}",
                                name=f"psum{m}{t}")

            # ---- x transposes into psum banks, then bottom MLP --------
            xT = apool.tile([13, nt * 512], f32r, tag="xT")
            for t in range(nt):
                ps = psum(0, t)
                for c in range(4):
                    nc.tensor.transpose(
                        out=ps[0:39, c * 128:(c + 1) * 128],
                        in_=x_sb[:].rearrange("p (j d) -> p j d", d=39)
                            [:, 4 * t + c, :],
                        identity=ident32[:])
                nc.vector.tensor_copy(out=xT[:, t * 512:(t + 1) * 512],
                                      in_=ps[0:13, :].bitcast(f32r))

            # activation helper: half on scalar ACT, half on DVE
            def act_relu(dst, src, bias_ap, scale=1.0, on_dve=False):
                if on_dve:
                    # DVE: out = max(src*scale + bias, 0) -- needs two-step
                    # when scale != 1; bias is [p, 1] broadcast.
                    if scale != 1.0:
                        nc.vector.tensor_scalar(
                            out=dst, in0=src, scalar1=float(scale),
                            scalar2=None, op0=mybir.AluOpType.mult)
                        nc.vector.tensor_scalar(
                            out=dst, in0=dst, scalar1=bias_ap, scalar2=0.0,
                            op0=mybir.AluOpType.add,
                            op1=mybir.AluOpType.max)
                    else:
                        nc.vector.tensor_scalar(
                            out=dst, in0=src, scalar1=bias_ap, scalar2=0.0,
                            op0=mybir.AluOpType.add,
                            op1=mybir.AluOpType.max)
                else:
                    nc.scalar.activation(dst, src, Relu, bias=bias_ap,
                                         scale=float(scale))

            # bottom L0: h1 = relu(bw0.T x + bb0); [128,(m,n)] f32r
            h1 = apool.tile([128, 2 * bc], f32r, tag="actA")
            for m in range(2):
                for t in range(nt):
                    ps = psum(m, t)
                    nc.tensor.matmul(out=ps[:],
                                     lhsT=bw0_sb[:, m * 128:(m + 1) * 128],
                                     rhs=xT[:, t * 512:(t + 1) * 512],
                                     start=True, stop=True,
                                     skip_group_check=True)
                    act_relu(h1[:, m * bc + t * 512: m * bc + (t + 1) * 512],
                             ps[:], biases["bb0"][:, m:m + 1],
                             on_dve=(m == 1))

            # bottom L1/L2 with residual: h_{l+1} = relu(W h + b) + h
            hprev = h1
            for li, (wnm, bnm, tg) in enumerate(
                    [("bw1", "bb1", "actB"), ("bw2", "bb2", "actC")]):
                hn = apool.tile([128, 2 * bc], f32r, tag=tg)
                for m in range(2):
                    for t in range(nt):
                        ps = psum(m, t)
                        for kk in range(2):
                            nc.tensor.matmul(
                                out=ps[:],
                                lhsT=mlp_w[wnm][:, kk, m * 128:(m + 1) * 128],
                                rhs=hprev[:, kk * bc + t * 512:
                                          kk * bc + (t + 1) * 512],
                                start=(kk == 0), stop=(kk == 1),
                                skip_group_check=True)
                        rl = rlpool.tile([128, 512], f32r, tag="rl", name="rl")
                        act_relu(rl[:], ps[:], biases[bnm][:, m:m + 1],
                                 on_dve=(m == 1))
                        nc.vector.tensor_add(
                            out=hn[:, m * bc + t * 512: m * bc + (t + 1) * 512],
                            in0=rl[:],
                            in1=hprev[:, m * bc + t * 512:
                                      m * bc + (t + 1) * 512])
                hprev = hn

            # ---- top L0: dense f32r (pre-scaled 2^16) + fp8 DoubleRow --
            # dense first (start=True), then pairs as transposes land.
            ps0 = {(m, t): psum(m, t) for m in range(2) for t in range(nt)}
            for m in range(2):
                for t in range(nt):
                    ps = ps0[(m, t)]
                    for kk in range(2):
                        nc.tensor.matmul(
                            out=ps[:],
                            lhsT=tw0h_sb[:].rearrange("p (k m) -> p k m", k=2)
                                [:, kk, m * 128:(m + 1) * 128],
                            rhs=hprev[:, kk * bc + t * 512:
                                      kk * bc + (t + 1) * 512],
                            start=(kk == 0), stop=False,
                            skip_group_check=True)
            w8v = tw0e_sb[:].rearrange("p (j i h m) -> p j i h m",
                                       j=NPAIR, i=2, h=2)
            for j in range(NPAIR):
                for m in range(2):
                    for t in range(nt):
                        nc.tensor.matmul(
                            out=ps0[(m, t)][:],
                            lhsT=w8v[:, j, :, m, :],
                            rhs=zts[j][:].bitcast(f8)
                                .rearrange("p (n two) -> p two n", two=2)
                                [:, :, t * 512:(t + 1) * 512],
                            start=False, stop=(j == NPAIR - 1),
                            perf_mode=mybir.MatmulPerfMode.DoubleRow,
                            skip_group_check=True)

            z0 = apool.tile([128, 2 * bc], f32r, tag="actA")
            for m in range(2):
                for t in range(nt):
                    act_relu(z0[:, m * bc + t * 512: m * bc + (t + 1) * 512],
                             ps0[(m, t)][:], biases["tb0"][:, m:m + 1],
                             scale=1.0 / PSCALE, on_dve=(m == 1))

            # ---- top L1-3 with residual -------------------------------
            zprev = z0
            for li, (wnm, bnm, tg) in enumerate(
                    [("tw1", "tb1", "actB"), ("tw2", "tb2", "actC"),
                     ("tw3", "tb3", "actA")]):
                zn = apool.tile([128, 2 * bc], f32r, tag=tg)
                for m in range(2):
                    for t in range(nt):
                        ps = psum(m, t)
                        for kk in range(2):
                            nc.tensor.matmul(
                                out=ps[:],
                                lhsT=mlp_w[wnm][:, kk, m * 128:(m + 1) * 128],
                                rhs=zprev[:, kk * bc + t * 512:
                                          kk * bc + (t + 1) * 512],
                                start=(kk == 0), stop=(kk == 1),
                                skip_group_check=True)
                        rl = rlpool.tile([128, 512], f32r, tag="rl", name="rl")
                        act_relu(rl[:], ps[:], biases[bnm][:, m:m + 1],
                                 on_dve=(m == 1))
                        nc.vector.tensor_add(
                            out=zn[:, m * bc + t * 512: m * bc + (t + 1) * 512],
                            in0=rl[:],
                            in1=zprev[:, m * bc + t * 512:
                                      m * bc + (t + 1) * 512])
                zprev = zn

            # ---- final layer [256 -> 1] -------------------------------
            yo = apool.tile([1, bc], f32, tag="yo")
            for t in range(nt):
                ps = psum(0, t)
                for kk in range(2):
                    nc.tensor.matmul(
                        out=ps[:1, :],
                        lhsT=mlp_w["tw4"][:, kk, 0:1],
                        rhs=zprev[:, kk * bc + t * 512:
                                  kk * bc + (t + 1) * 512],
                        start=(kk == 0), stop=(kk == 1),
                        skip_group_check=True)
                nc.scalar.activation(yo[:, t * 512:(t + 1) * 512], ps[:1, :],
                                     Ident, bias=biases["tb4"][:, 0:1])
            nc.sync.dma_start(out=out[:].rearrange("s o -> o s"), in_=yo[:])

    if split_waits:
        _split_multi_waits(nc)
    return nc


_NC_CACHE = {}


def _get_nc(bc=BC, v=V):
    key = (bc, v)
    if key not in _NC_CACHE:
        _NC_CACHE[key] = build_bass(bc=bc, v=v)
    return _NC_CACHE[key]


def make_in_maps(inputs, bc=BC, v=V, n_cores=N_CORES):
    x = np.ascontiguousarray(np.asarray(inputs["x"], dtype=np.float32))
    table = np.concatenate([np.asarray(inputs[f"emb{i}"]) for i in range(4)],
                           axis=0)
    assert table.shape[0] == v
    table8 = (table * np.float32(TSCALE)).astype(ml_dtypes.float8_e4m3)
    tw0 = np.asarray(inputs["tw0"], dtype=np.float32)
    tw0e = tw0[256:]                       # [26*128, 256]
    # pair layout: w8[p, j, i, h, m'] = tw0e[(2j + (p>=64))*128 + 2*(p%64)+i,
    #                                        128*h + m'] * WSCALE
    w = tw0e.reshape(NSLOT, D, 2, 128)     # [slot, d, h, m']
    w8 = np.empty((128, NPAIR, 2, 2, 128), np.float32)
    p = np.arange(128)
    for j in range(NPAIR):
        for i in range(2):
            # slot = 2j + (p>=64); d = 2*(p%64) + i
            w8[:, j, i, :, :] = w[2 * j + (p >= 64).astype(int),
                                  2 * (p % 64) + i] * np.float32(WSCALE)
    w8 = w8.reshape(128, -1).astype(ml_dtypes.float8_e4m3)
    shared = {
        "table": table8,
        "tw0e8": w8,
        "tw0h": np.ascontiguousarray(tw0[:256]) * np.float32(PSCALE),
    }
    for i in range(3):
        shared[f"bw{i}"] = np.asarray(inputs[f"bw{i}"], dtype=np.float32)
        shared[f"bb{i}"] = np.asarray(
            inputs[f"bb{i}"], dtype=np.float32).reshape(-1, 1)
    for i in range(5):
        if i > 0:
            shared[f"tw{i}"] = np.asarray(inputs[f"tw{i}"], dtype=np.float32)
        shared[f"tb{i}"] = np.asarray(
            inputs[f"tb{i}"], dtype=np.float32).reshape(-1, 1)
    return [dict(shared, xs=np.ascontiguousarray(x[c * bc:(c + 1) * bc]))
            for c in range(n_cores)]


LAST_EXEC_NS = None
LAST_RES = None


def _dump_trace(res, path="/tmp/last_trace.json"):
    import json
    if not res.instructions_and_trace:
        return
    insts, tp = res.instructions_and_trace
    rows = []
    for it in insts:
        try:
            rows.append({
                "name": it.name, "op": it.op_name, "engine": str(it.engine),
                "ts": it.timestamp, "dur": it.duration,
                "end": it.end_timestamp,
                "label": getattr(it, "label", None),
                "queue": getattr(it, "queue", None),
            })
        except Exception:
            pass
    with open(path, "w") as f:
        json.dump({"trace_path": tp, "insts": rows}, f)


def kernel(**inputs) -> np.ndarray:
    global LAST_EXEC_NS, LAST_RES
    nc = _get_nc()
    in_maps = make_in_maps(inputs)
    trace = bool(int(os.environ.get("KERNEL_TRACE", "0")))
    res = run_bass_kernel_spmd(nc, in_maps, core_ids=list(range(N_CORES)),
                               trace=trace)
    LAST_EXEC_NS = res.exec_time_ns
    LAST_RES = res
    if trace:
        try:
            _dump_trace(res)
        except Exception as e:
            print(f"(trace dump failed: {e})")
    return np.concatenate([res.results[c]["out"] for c in range(N_CORES)],
                          axis=0)



# revision 3
# speedup vs baseline: 9.0553x; 9.0553x over previous
"""DLRM-ResNet forward on 8 TRN2 cores — v3.

Data-parallel over batch (2048 samples/core); embedding table replicated
per core in fp8-e4m3 (values are uniform [0, 2^-11]; stored pre-scaled by
2^11 so they quantize well; the 2^-11 is folded into the top-MLP layer-0
scales). Per core:

  - 13 batched indirect gathers (one per slot-pair: 32 indices/partition,
    512 KB each) on the Pool queue. v2 used 416 per-subtile gathers; the
    ~1 us fixed SWDGE cost per instruction made the gather stream the
    whole-kernel critical path.
  - 13 batched XBAR DMA-transposes (one per pair; 3D out AP [q, jj, s]
    gives the blockwise transpose in a single instruction), alternating
    between the two HWDGE queues (sync / scalar). The u16 view of the
    gathered fp8 tile lands d-pairs on partitions: slot 2j -> partitions
    0..63 of pair tile j, slot 2j+1 -> 64..127.
  - Embedding part of top-MLP layer 0 runs as fp8 DoubleRow matmuls
    (2 rows/cycle): lhsT [128, 2, 128] member-major, rhs [128, 2, 512]
    byte-strided view of the pair tile.
  - MLP weights/activations in bf16 (2x DVE throughput, FWL weight
    loads); dense part of top-L0 pre-scaled 2^16 (= 2^11 table x 2^5
    fp8-weight scale) accumulates into the same PSUM banks; one
    activation applies scale 2^-16 + bias. Square weights ship
    host-prepacked in one [128, 3074] bf16 array (single DMA); biases in
    one [128, 15] f32 array.
  - ReLU+bias split between scalar ACT and DVE tensor_scalar; residual
    adds on DVE in bf16.
"""

import os
import numpy as np
import ml_dtypes

import concourse.bass as bass
import concourse.tile as tile
import concourse.mybir as mybir
from concourse.bass_utils import run_bass_kernel_spmd
from concourse.masks import make_identity

f8 = mybir.dt.float8e4
bf16 = mybir.dt.bfloat16
f32 = mybir.dt.float32
u16 = mybir.dt.uint16
i32 = mybir.dt.int32

B, V, D = 16384, 4194304, 128
N_CORES = 8
BC = B // N_CORES          # 2048 samples per core
NSLOT = 26
NPAIR = 13
BOT = [13, 256, 256, 256]

TSCALE = 2.0 ** 11         # table pre-scale
WSCALE = 2.0 ** 5          # tw0e fp8 pre-scale
PSCALE = TSCALE * WSCALE   # dense-path pre-scale / activation inverse scale

Relu = mybir.ActivationFunctionType.Relu
Ident = mybir.ActivationFunctionType.Identity

# host-prepacked [128, (k m)] bf16 square-weight array layout
WSQ = [("tw0h", 512), ("bw1", 512), ("bw2", 512), ("tw1", 512),
       ("tw2", 512), ("tw3", 512), ("tw4", 2)]
WSQ_COLS = sum(c for _, c in WSQ)
BIAS_ORDER = ["bb0", "bb1", "bb2", "tb0", "tb1", "tb2", "tb3"]  # 2 cols each
# tb4 is col 14 (scalar); total 15 cols


def _split_multi_waits(nc):
    """walrus accepts only ONE sync-wait per instruction; hoist extras onto
    same-engine EventSemaphore instructions immediately before."""
    n_new = 0
    for fn in nc.m.functions:
        for bb in fn.blocks:
            insts = list(bb.instructions)
            out = []
            changed = False
            for inst in insts:
                si = inst.sync_info
                waits = list(si.on_wait) if si is not None else []
                if len(waits) > 1:
                    changed = True
                    for w in waits[:-1]:
                        n_new += 1
                        out.append(mybir.InstEventSemaphore(
                            name=f"I-ws-{n_new}",
                            engine=inst.engine,
                            ins=[], outs=[],
                            sync_info=mybir.SyncInfo(on_wait=[w], on_update=[]),
                        ))
                    si.on_wait = [waits[-1]]
                out.append(inst)
            if changed:
                bb.instructions = out
    return n_new


def build_bass(bc=BC, v=V, split_waits=True):
    nc = bass.Bass("TRN2", target_bir_lowering=False, debug=False)
    nj = bc // 128
    nt = bc // 512

    xs = nc.dram_tensor("xs", [bc, 39], f32, kind="ExternalInput")
    table = nc.dram_tensor("table", [v, D], f8, kind="ExternalInput")
    # pair-layout fp8 weights: [p][pair, member, mhalf, m']
    tw0e8 = nc.dram_tensor("tw0e8", [128, NPAIR * 2 * 2 * 128], f8,
                           kind="ExternalInput")
    wsq = nc.dram_tensor("wsq", [128, WSQ_COLS], bf16, kind="ExternalInput")
    bw0d = nc.dram_tensor("bw0p", [13, 256], bf16, kind="ExternalInput")
    biasd = nc.dram_tensor("biasp", [128, 15], f32, kind="ExternalInput")
    out = nc.dram_tensor("out", [bc, 1], f32, kind="ExternalOutput")

    with tile.TileContext(nc) as tc:
        with tc.tile_pool(name="const", bufs=1) as cpool, \
             tc.tile_pool(name="wpool", bufs=1) as wpool, \
             tc.tile_pool(name="epool", bufs=3) as epool, \
             tc.tile_pool(name="ztp", bufs=1) as ztpool, \
             tc.tile_pool(name="act", bufs=1) as apool, \
             tc.tile_pool(name="rlp", bufs=2) as rlpool, \
             tc.tile_pool(name="psb", bufs=1, space="PSUM") as psp:

            ident32 = cpool.tile([128, 128], f32)
            make_identity(nc, ident32[:])

            # ---- x load (sample s = 128*j + p -> partition p, block j) ----
            x_sb = cpool.tile([128, nj * 39], f32)
            nc.sync.dma_start(
                out=x_sb[:].rearrange("p (j d) -> p j d", d=39),
                in_=xs[:].rearrange("(j p) d -> p j d", p=128))

            # ---- indices: [p, (j, jj, a)] — matches e_t block order ------
            idx_f = cpool.tile([128, NSLOT * nj], f32)
            idx_i = cpool.tile([128, NSLOT * nj], i32)
            idx_m = cpool.tile([128, NSLOT * nj], i32)
            nc.vector.tensor_copy(
                out=idx_f[:].rearrange("p (j jj a) -> p j jj a",
                                       j=NPAIR, a=2),
                in_=x_sb[:].rearrange("p (jj d) -> p jj d", d=39)[:, :, 13:39]
                    .rearrange("p jj (j a) -> p j jj a", a=2))
            nc.vector.tensor_copy(out=idx_i[:], in_=idx_f[:])
            nc.vector.tensor_scalar(
                out=idx_m[:], in0=idx_i[:], scalar1=v - 1, scalar2=None,
                op0=mybir.AluOpType.bitwise_and)

            # ---- weights (single DMA each; host-prepacked) ---------------
            tw0e_sb = wpool.tile([128, NPAIR * 2 * 2 * 128], f8)
            nc.sync.dma_start(out=tw0e_sb[:], in_=tw0e8[:])
            wsq_sb = wpool.tile([128, WSQ_COLS], bf16)
            nc.sync.dma_start(out=wsq_sb[:], in_=wsq[:])
            bw0_sb = wpool.tile([13, 256], bf16)
            nc.sync.dma_start(out=bw0_sb[:], in_=bw0d[:])
            bias_sb = wpool.tile([128, 15], f32)
            nc.sync.dma_start(out=bias_sb[:], in_=biasd[:])

            mlp_w = {}
            off = 0
            for nm, cols in WSQ:
                k = 2
                mlp_w[nm] = wsq_sb[:, off:off + cols].rearrange(
                    "p (k m) -> p k m", k=k)
                off += cols

            def bias_ap(nm, m=0):
                if nm == "tb4":
                    return bias_sb[0:1, 14:15]
                c = BIAS_ORDER.index(nm) * 2 + m
                return bias_sb[:, c:c + 1]

            # ---- batched gathers + batched transposes per pair ----------
            zts = [ztpool.tile([128, nj * 128], u16, tag=f"zt{j}",
                               name=f"zt{j}")
                   for j in range(NPAIR)]
            for j in range(NPAIR):
                e_t = epool.tile([128, 2 * nj * 128], f8, tag="ep")
                nc.gpsimd.indirect_dma_start(
                    out=e_t[:],
                    out_offset=None,
                    in_=table[:],
                    in_offset=bass.IndirectOffsetOnAxis(
                        ap=idx_m[:, j * 2 * nj:(j + 1) * 2 * nj], axis=0))
                eng = nc.sync if j % 2 == 0 else nc.scalar
                eng.dma_start_transpose(
                    out=zts[j][:].rearrange("q (jj s) -> q jj s", s=128),
                    in_=e_t[:].bitcast(u16))

            # ---- PSUM banks: tag ps{m}{t} -----------------------------
            def psum(m, t):
                return psp.tile([128, 512], f32, space="PSUM", tag=f"ps{m}{t}",
                                name=f"psum{m}{t}")

            # ---- x transposes into psum banks, then bottom MLP --------
            xT = apool.tile([13, nt * 512], bf16, tag="xT")
            for t in range(nt):
                ps = psum(0, t)
                for c in range(4):
                    nc.tensor.transpose(
                        out=ps[0:39, c * 128:(c + 1) * 128],
                        in_=x_sb[:].rearrange("p (j d) -> p j d", d=39)
                            [:, 4 * t + c, :],
                        identity=ident32[:])
                nc.vector.tensor_copy(out=xT[:, t * 512:(t + 1) * 512],
                                      in_=ps[0:13, :])

            # activation helper: half on scalar ACT, half on DVE
            def act_relu(dst, src, b_ap, scale=1.0, on_dve=False):
                if on_dve:
                    if scale != 1.0:
                        nc.vector.tensor_scalar(
                            out=dst, in0=src, scalar1=float(scale),
                            scalar2=None, op0=mybir.AluOpType.mult)
                        nc.vector.tensor_scalar(
                            out=dst, in0=dst, scalar1=b_ap, scalar2=0.0,
                            op0=mybir.AluOpType.add,
                            op1=mybir.AluOpType.max)
                    else:
                        nc.vector.tensor_scalar(
                            out=dst, in0=src, scalar1=b_ap, scalar2=0.0,
                            op0=mybir.AluOpType.add,
                            op1=mybir.AluOpType.max)
                else:
                    nc.scalar.activation(dst, src, Relu, bias=b_ap,
                                         scale=float(scale))

            # bottom L0: h1 = relu(bw0.T x + bb0); [128,(m,n)] bf16
            h1 = apool.tile([128, 2 * bc], bf16, tag="actA")
            for m in range(2):
                for t in range(nt):
                    ps = psum(m, t)
                    nc.tensor.matmul(out=ps[:],
                                     lhsT=bw0_sb[:, m * 128:(m + 1) * 128],
                                     rhs=xT[:, t * 512:(t + 1) * 512],
                                     start=True, stop=True,
                                     skip_group_check=True)
                    act_relu(h1[:, m * bc + t * 512: m * bc + (t + 1) * 512],
                             ps[:], bias_ap("bb0", m),
                             on_dve=(m == 1))

            # bottom L1/L2 with residual: h_{l+1} = relu(W h + b) + h
            hprev = h1
            for li, (wnm, bnm, tg) in enumerate(
                    [("bw1", "bb1", "actB"), ("bw2", "bb2", "actC")]):
                hn = apool.tile([128, 2 * bc], bf16, tag=tg)
                for m in range(2):
                    for t in range(nt):
                        ps = psum(m, t)
                        for kk in range(2):
                            nc.tensor.matmul(
                                out=ps[:],
                                lhsT=mlp_w[wnm][:, kk, m * 128:(m + 1) * 128],
                                rhs=hprev[:, kk * bc + t * 512:
                                          kk * bc + (t + 1) * 512],
                                start=(kk == 0), stop=(kk == 1),
                                skip_group_check=True)
                        rl = rlpool.tile([128, 512], bf16, tag="rl", name="rl")
                        act_relu(rl[:], ps[:], bias_ap(bnm, m),
                                 on_dve=(m == 1))
                        nc.vector.tensor_add(
                            out=hn[:, m * bc + t * 512: m * bc + (t + 1) * 512],
                            in0=rl[:],
                            in1=hprev[:, m * bc + t * 512:
                                      m * bc + (t + 1) * 512])
                hprev = hn

            # ---- top L0: dense bf16 (pre-scaled 2^16) + fp8 DoubleRow --
            # dense first (start=True), then pairs as transposes land.
            ps0 = {(m, t): psum(m, t) for m in range(2) for t in range(nt)}
            for m in range(2):
                for t in range(nt):
                    ps = ps0[(m, t)]
                    for kk in range(2):
                        nc.tensor.matmul(
                            out=ps[:],
                            lhsT=mlp_w["tw0h"][:, kk, m * 128:(m + 1) * 128],
                            rhs=hprev[:, kk * bc + t * 512:
                                      kk * bc + (t + 1) * 512],
                            start=(kk == 0), stop=False,
                            skip_group_check=True)
            w8v = tw0e_sb[:].rearrange("p (j i h m) -> p j i h m",
                                       j=NPAIR, i=2, h=2)
            for j in range(NPAIR):
                for m in range(2):
                    for t in range(nt):
                        nc.tensor.matmul(
                            out=ps0[(m, t)][:],
                            lhsT=w8v[:, j, :, m, :],
                            rhs=zts[j][:].bitcast(f8)
                                .rearrange("p (n two) -> p two n", two=2)
                                [:, :, t * 512:(t + 1) * 512],
                            start=False, stop=(j == NPAIR - 1),
                            perf_mode=mybir.MatmulPerfMode.DoubleRow,
                            skip_group_check=True)

            z0 = apool.tile([128, 2 * bc], bf16, tag="actA")
            for m in range(2):
                for t in range(nt):
                    act_relu(z0[:, m * bc + t * 512: m * bc + (t + 1) * 512],
                             ps0[(m, t)][:], bias_ap("tb0", m),
                             scale=1.0 / PSCALE, on_dve=(m == 1))

            # ---- top L1-3 with residual -------------------------------
            zprev = z0
            for li, (wnm, bnm, tg) in enumerate(
                    [("tw1", "tb1", "actB"), ("tw2", "tb2", "actC"),
                     ("tw3", "tb3", "actA")]):
                zn = apool.tile([128, 2 * bc], bf16, tag=tg)
                for m in range(2):
                    for t in range(nt):
                        ps = psum(m, t)
                        for kk in range(2):
                            nc.tensor.matmul(
                                out=ps[:],
                                lhsT=mlp_w[wnm][:, kk, m * 128:(m + 1) * 128],
                                rhs=zprev[:, kk * bc + t * 512:
                                          kk * bc + (t + 1) * 512],
                                start=(kk == 0), stop=(kk == 1),
                                skip_group_check=True)
                        rl = rlpool.tile([128, 512], bf16, tag="rl", name="rl")
                        act_relu(rl[:], ps[:], bias_ap(bnm, m),
                                 on_dve=(m == 1))
                        nc.vector.tensor_add(
                            out=zn[:, m * bc + t * 512: m * bc + (t + 1) * 512],
                            in0=rl[:],
                            in1=zprev[:, m * bc + t * 512:
                                      m * bc + (t + 1) * 512])
                zprev = zn

            # ---- final layer [256 -> 1] -------------------------------
            yo = apool.tile([1, bc], f32, tag="yo")
            for t in range(nt):
                ps = psum(0, t)
                for kk in range(2):
                    nc.tensor.matmul(
                        out=ps[:1, :],
                        lhsT=mlp_w["tw4"][:, kk, 0:1],
                        rhs=zprev[:, kk * bc + t * 512:
                                  kk * bc + (t + 1) * 512],
                        start=(kk == 0), stop=(kk == 1),
                        skip_group_check=True)
                nc.scalar.activation(yo[:, t * 512:(t + 1) * 512], ps[:1, :],
                                     Ident, bias=bias_ap("tb4"))
            nc.sync.dma_start(out=out[:].rearrange("s o -> o s"), in_=yo[:])

    if split_waits:
        _split_multi_waits(nc)
    return nc


_NC_CACHE = {}


def _get_nc(bc=BC, v=V):
    key = (bc, v)
    if key not in _NC_CACHE:
        _NC_CACHE[key] = build_bass(bc=bc, v=v)
    return _NC_CACHE[key]


def make_in_maps(inputs, bc=BC, v=V, n_cores=N_CORES):
    bf = ml_dtypes.bfloat16
    x = np.ascontiguousarray(np.asarray(inputs["x"], dtype=np.float32))
    table = np.concatenate([np.asarray(inputs[f"emb{i}"]) for i in range(4)],
                           axis=0)
    assert table.shape[0] == v
    table8 = (table * np.float32(TSCALE)).astype(ml_dtypes.float8_e4m3)
    tw0 = np.asarray(inputs["tw0"], dtype=np.float32)
    tw0e = tw0[256:]                       # [26*128, 256]
    # pair layout: w8[p, j, i, h, m'] = tw0e[(2j + (p>=64))*128 + 2*(p%64)+i,
    #                                        128*h + m'] * WSCALE
    w = tw0e.reshape(NSLOT, D, 2, 128)     # [slot, d, h, m']
    w8 = np.empty((128, NPAIR, 2, 2, 128), np.float32)
    p = np.arange(128)
    for j in range(NPAIR):
        for i in range(2):
            # slot = 2j + (p>=64); d = 2*(p%64) + i
            w8[:, j, i, :, :] = w[2 * j + (p >= 64).astype(int),
                                  2 * (p % 64) + i] * np.float32(WSCALE)
    w8 = w8.reshape(128, -1).astype(ml_dtypes.float8_e4m3)

    def pack_km(wm):                       # [256, fo] -> [128, 2*fo]
        fo = wm.shape[1]
        return np.ascontiguousarray(
            wm.reshape(2, 128, fo).transpose(1, 0, 2).reshape(128, 2 * fo))

    sq = {
        "tw0h": pack_km(tw0[:256] * np.float32(PSCALE)),
        "bw1": pack_km(np.asarray(inputs["bw1"], dtype=np.float32)),
        "bw2": pack_km(np.asarray(inputs["bw2"], dtype=np.float32)),
        "tw1": pack_km(np.asarray(inputs["tw1"], dtype=np.float32)),
        "tw2": pack_km(np.asarray(inputs["tw2"], dtype=np.float32)),
        "tw3": pack_km(np.asarray(inputs["tw3"], dtype=np.float32)),
        "tw4": pack_km(np.asarray(inputs["tw4"], dtype=np.float32)),
    }
    wsq = np.concatenate([sq[nm] for nm, _ in WSQ], axis=1).astype(bf)
    assert wsq.shape == (128, WSQ_COLS)

    biasp = np.zeros((128, 15), np.float32)
    for bi, nm in enumerate(BIAS_ORDER):
        b = np.asarray(inputs[nm], dtype=np.float32).reshape(-1)
        biasp[:, 2 * bi] = b[:128]
        biasp[:, 2 * bi + 1] = b[128:]
    biasp[0, 14] = np.float32(np.asarray(inputs["tb4"]).reshape(-1)[0])

    shared = {
        "table": table8,
        "tw0e8": w8,
        "wsq": wsq,
        "bw0p": np.asarray(inputs["bw0"], dtype=np.float32).astype(bf),
        "biasp": biasp,
    }
    return [dict(shared, xs=np.ascontiguousarray(x[c * bc:(c + 1) * bc]))
            for c in range(n_cores)]


LAST_EXEC_NS = None
LAST_RES = None


def _dump_trace(res, path="/tmp/last_trace.json"):
    import json
    if not res.instructions_and_trace:
        return
    insts, tp = res.instructions_and_trace
    rows = []
    for it in insts:
        try:
            rows.append({
                "name": it.name, "op": it.op_name, "engine": str(it.engine),
                "ts": it.timestamp, "dur": it.duration,
                "end": it.end_timestamp,
            })
        except Exception:
            pass
    with open(path, "w") as f:
        json.dump({"trace_path": tp, "insts": rows}, f)


def kernel(**inputs) -> np.ndarray:
    global LAST_EXEC_NS, LAST_RES
    nc = _get_nc()
    in_maps = make_in_maps(inputs)
    trace = bool(int(os.environ.get("KERNEL_TRACE", "0")))
    res = run_bass_kernel_spmd(nc, in_maps, core_ids=list(range(N_CORES)),
                               trace=trace)
    LAST_EXEC_NS = res.exec_time_ns
    LAST_RES = res
    if trace:
        try:
            _dump_trace(res)
        except Exception as e:
            print(f"(trace dump failed: {e})")
    return np.concatenate([res.results[c]["out"] for c in range(N_CORES)],
                          axis=0)


# revision 7
# speedup vs baseline: 9.3930x; 1.0373x over previous
"""DLRM-ResNet forward on 8 TRN2 cores — v3.

Data-parallel over batch (2048 samples/core); embedding table replicated
per core in fp8-e4m3 (values are uniform [0, 2^-11]; stored pre-scaled by
2^11 so they quantize well; the 2^-11 is folded into the top-MLP layer-0
scales). Per core:

  - 13 batched indirect gathers (one per slot-pair: 32 indices/partition,
    512 KB each) on the Pool queue. v2 used 416 per-subtile gathers; the
    ~1 us fixed SWDGE cost per instruction made the gather stream the
    whole-kernel critical path.
  - 13 batched XBAR DMA-transposes (one per pair; 3D out AP [q, jj, s]
    gives the blockwise transpose in a single instruction), alternating
    between the two HWDGE queues (sync / scalar). The u16 view of the
    gathered fp8 tile lands d-pairs on partitions: slot 2j -> partitions
    0..63 of pair tile j, slot 2j+1 -> 64..127.
  - Embedding part of top-MLP layer 0 runs as fp8 DoubleRow matmuls
    (2 rows/cycle): lhsT [128, 2, 128] member-major, rhs [128, 2, 512]
    byte-strided view of the pair tile.
  - MLP weights/activations in bf16 (2x DVE throughput, FWL weight
    loads); dense part of top-L0 pre-scaled 2^16 (= 2^11 table x 2^5
    fp8-weight scale) accumulates into the same PSUM banks; one
    activation applies scale 2^-16 + bias. Square weights ship
    host-prepacked in one [128, 3074] bf16 array (single DMA); biases in
    one [128, 15] f32 array.
  - ReLU+bias split between scalar ACT and DVE tensor_scalar; residual
    adds on DVE in bf16.
"""

import os
import numpy as np
import ml_dtypes

import concourse.bass as bass
import concourse.tile as tile
import concourse.mybir as mybir
from concourse.bass_utils import run_bass_kernel_spmd
from concourse.masks import make_identity

f8 = mybir.dt.float8e4
bf16 = mybir.dt.bfloat16
f32 = mybir.dt.float32
u16 = mybir.dt.uint16
i32 = mybir.dt.int32

B, V, D = 16384, 4194304, 128
N_CORES = 8
BC = B // N_CORES          # 2048 samples per core
NSLOT = 26
NPAIR = 13
BOT = [13, 256, 256, 256]

TSCALE = 2.0 ** 11         # table pre-scale
WSCALE = 2.0 ** 5          # tw0e fp8 pre-scale
PSCALE = TSCALE * WSCALE   # dense-path pre-scale / activation inverse scale

Relu = mybir.ActivationFunctionType.Relu
Ident = mybir.ActivationFunctionType.Identity

# host-prepacked [128, (k m)] bf16 square-weight array layout
WSQ = [("tw0h", 512), ("bw1", 512), ("bw2", 512), ("tw1", 512),
       ("tw2", 512), ("tw3", 512), ("tw4", 2)]
WSQ_COLS = sum(c for _, c in WSQ)
BIAS_ORDER = ["bb0", "bb1", "bb2", "tb0", "tb1", "tb2", "tb3"]  # 2 cols each
# tb4 is col 14 (scalar); total 15 cols


def _split_multi_waits(nc):
    """walrus accepts only ONE sync-wait per instruction; hoist extras onto
    same-engine EventSemaphore instructions immediately before."""
    n_new = 0
    for fn in nc.m.functions:
        for bb in fn.blocks:
            insts = list(bb.instructions)
            out = []
            changed = False
            for inst in insts:
                si = inst.sync_info
                waits = list(si.on_wait) if si is not None else []
                if len(waits) > 1:
                    changed = True
                    for w in waits[:-1]:
                        n_new += 1
                        out.append(mybir.InstEventSemaphore(
                            name=f"I-ws-{n_new}",
                            engine=inst.engine,
                            ins=[], outs=[],
                            sync_info=mybir.SyncInfo(on_wait=[w], on_update=[]),
                        ))
                    si.on_wait = [waits[-1]]
                out.append(inst)
            if changed:
                bb.instructions = out
    return n_new


def build_bass(bc=BC, v=V, split_waits=True):
    nc = bass.Bass("TRN2", target_bir_lowering=False, debug=False)
    nj = bc // 128
    nt = bc // 512

    xs = nc.dram_tensor("xs", [bc, 39], f32, kind="ExternalInput")
    table = nc.dram_tensor("table", [v, D], f8, kind="ExternalInput")
    # pair-layout fp8 weights: [p][pair, member, mhalf, m']
    tw0e8 = nc.dram_tensor("tw0e8", [128, NPAIR * 2 * 2 * 128], f8,
                           kind="ExternalInput")
    wsq = nc.dram_tensor("wsq", [128, WSQ_COLS], bf16, kind="ExternalInput")
    bw0d = nc.dram_tensor("bw0p", [13, 256], bf16, kind="ExternalInput")
    biasd = nc.dram_tensor("biasp", [128, 15], f32, kind="ExternalInput")
    out = nc.dram_tensor("out", [bc, 1], f32, kind="ExternalOutput")

    with tile.TileContext(nc) as tc:
        with tc.tile_pool(name="const", bufs=1) as cpool, \
             tc.tile_pool(name="wpool", bufs=1) as wpool, \
             tc.tile_pool(name="epool", bufs=5) as epool, \
             tc.tile_pool(name="ztp", bufs=1) as ztpool, \
             tc.tile_pool(name="act", bufs=1) as apool, \
             tc.tile_pool(name="rlp", bufs=2) as rlpool, \
             tc.tile_pool(name="psb", bufs=1, space="PSUM") as psp:

            ident32 = cpool.tile([128, 128], f32)
            make_identity(nc, ident32[:])

            # ---- x load (sample s = 128*j + p -> partition p, block j) ----
            x_sb = cpool.tile([128, nj * 39], f32)
            nc.sync.dma_start(
                out=x_sb[:].rearrange("p (j d) -> p j d", d=39),
                in_=xs[:].rearrange("(j p) d -> p j d", p=128))

            # ---- indices: [p, (j, jj, a)] — matches e_t block order ------
            idx_i = cpool.tile([128, NSLOT * nj], i32)
            idx_m = cpool.tile([128, NSLOT * nj], i32)
            nc.vector.tensor_copy(
                out=idx_i[:].rearrange("p (j jj a) -> p j jj a",
                                       j=NPAIR, a=2),
                in_=x_sb[:].rearrange("p (jj d) -> p jj d", d=39)[:, :, 13:39]
                    .rearrange("p jj (j a) -> p j jj a", a=2))
            nc.vector.tensor_scalar(
                out=idx_m[:], in0=idx_i[:], scalar1=v - 1, scalar2=None,
                op0=mybir.AluOpType.bitwise_and)

            # ---- weights (single DMA each; host-prepacked) ---------------
            tw0e_sb = wpool.tile([128, NPAIR * 2 * 2 * 128], f8)
            nc.sync.dma_start(out=tw0e_sb[:], in_=tw0e8[:])
            wsq_sb = wpool.tile([128, WSQ_COLS], bf16)
            nc.sync.dma_start(out=wsq_sb[:], in_=wsq[:])
            bw0_sb = wpool.tile([13, 256], bf16)
            nc.scalar.dma_start(out=bw0_sb[:], in_=bw0d[:])
            bias_sb = wpool.tile([128, 15], f32)
            nc.scalar.dma_start(out=bias_sb[:], in_=biasd[:])

            mlp_w = {}
            off = 0
            for nm, cols in WSQ:
                k = 2
                mlp_w[nm] = wsq_sb[:, off:off + cols].rearrange(
                    "p (k m) -> p k m", k=k)
                off += cols

            def bias_ap(nm, m=0):
                if nm == "tb4":
                    return bias_sb[0:1, 14:15]
                c = BIAS_ORDER.index(nm) * 2 + m
                return bias_sb[:, c:c + 1]

            # ---- batched gathers + batched transposes per pair ----------
            zts = [ztpool.tile([128, nj * 128], u16, tag=f"zt{j}",
                               name=f"zt{j}")
                   for j in range(NPAIR)]
            for j in range(NPAIR):
                e_t = epool.tile([128, 2 * nj * 128], f8, tag="ep")
                nc.gpsimd.indirect_dma_start(
                    out=e_t[:],
                    out_offset=None,
                    in_=table[:],
                    in_offset=bass.IndirectOffsetOnAxis(
                        ap=idx_m[:, j * 2 * nj:(j + 1) * 2 * nj], axis=0))
                nc.sync.dma_start_transpose(
                    out=zts[j][:].rearrange("q (jj s) -> q jj s", s=128),
                    in_=e_t[:].bitcast(u16))

            # ---- PSUM banks: tag ps{m}{t} -----------------------------
            def psum(m, t):
                return psp.tile([128, 512], f32, space="PSUM", tag=f"ps{m}{t}",
                                name=f"psum{m}{t}")

            # ---- x transposes into psum banks, then bottom MLP --------
            xT = apool.tile([13, nt * 512], bf16, tag="xT")
            for t in range(nt):
                ps = psum(0, t)
                for c in range(4):
                    nc.tensor.transpose(
                        out=ps[0:39, c * 128:(c + 1) * 128],
                        in_=x_sb[:].rearrange("p (j d) -> p j d", d=39)
                            [:, 4 * t + c, :],
                        identity=ident32[:])
                nc.vector.tensor_copy(out=xT[:, t * 512:(t + 1) * 512],
                                      in_=ps[0:13, :])

            # activation helper: half on scalar ACT, half on DVE
            def act_relu(dst, src, b_ap, scale=1.0, on_dve=False):
                if on_dve:
                    if scale != 1.0:
                        nc.vector.tensor_scalar(
                            out=dst, in0=src, scalar1=float(scale),
                            scalar2=None, op0=mybir.AluOpType.mult)
                        nc.vector.tensor_scalar(
                            out=dst, in0=dst, scalar1=b_ap, scalar2=0.0,
                            op0=mybir.AluOpType.add,
                            op1=mybir.AluOpType.max)
                    else:
                        nc.vector.tensor_scalar(
                            out=dst, in0=src, scalar1=b_ap, scalar2=0.0,
                            op0=mybir.AluOpType.add,
                            op1=mybir.AluOpType.max)
                else:
                    nc.scalar.activation(dst, src, Relu, bias=b_ap,
                                         scale=float(scale))

            # bottom L0: h1 = relu(bw0.T x + bb0); [128,(m,n)] bf16
            h1 = apool.tile([128, 2 * bc], bf16, tag="actA")
            for m in range(2):
                for t in range(nt):
                    ps = psum(m, t)
                    nc.tensor.matmul(out=ps[:],
                                     lhsT=bw0_sb[:, m * 128:(m + 1) * 128],
                                     rhs=xT[:, t * 512:(t + 1) * 512],
                                     start=True, stop=True,
                                     skip_group_check=True)
                    act_relu(h1[:, m * bc + t * 512: m * bc + (t + 1) * 512],
                             ps[:], bias_ap("bb0", m),
                             on_dve=(m == 1))

            # bottom L1/L2 with residual: h_{l+1} = relu(W h + b) + h
            hprev = h1
            for li, (wnm, bnm, tg) in enumerate(
                    [("bw1", "bb1", "actB"), ("bw2", "bb2", "actC")]):
                hn = apool.tile([128, 2 * bc], bf16, tag=tg)
                for m in range(2):
                    for t in range(nt):
                        ps = psum(m, t)
                        for kk in range(2):
                            nc.tensor.matmul(
                                out=ps[:],
                                lhsT=mlp_w[wnm][:, kk, m * 128:(m + 1) * 128],
                                rhs=hprev[:, kk * bc + t * 512:
                                          kk * bc + (t + 1) * 512],
                                start=(kk == 0), stop=(kk == 1),
                                skip_group_check=True)
                        rl = rlpool.tile([128, 512], bf16, tag="rl", name="rl")
                        act_relu(rl[:], ps[:], bias_ap(bnm, m),
                                 on_dve=(m == 1))
                        nc.vector.tensor_add(
                            out=hn[:, m * bc + t * 512: m * bc + (t + 1) * 512],
                            in0=rl[:],
                            in1=hprev[:, m * bc + t * 512:
                                      m * bc + (t + 1) * 512])
                hprev = hn

            # ---- top L0: dense bf16 (pre-scaled 2^16) + fp8 DoubleRow --
            # dense first (start=True), then pairs as transposes land.
            ps0 = {(m, t): psum(m, t) for m in range(2) for t in range(nt)}
            for m in range(2):
                for t in range(nt):
                    ps = ps0[(m, t)]
                    for kk in range(2):
                        nc.tensor.matmul(
                            out=ps[:],
                            lhsT=mlp_w["tw0h"][:, kk, m * 128:(m + 1) * 128],
                            rhs=hprev[:, kk * bc + t * 512:
                                      kk * bc + (t + 1) * 512],
                            start=(kk == 0), stop=False,
                            skip_group_check=True)
            w8v = tw0e_sb[:].rearrange("p (j i h m) -> p j i h m",
                                       j=NPAIR, i=2, h=2)
            for j in range(NPAIR):
                for m in range(2):
                    for t in range(nt):
                        nc.tensor.matmul(
                            out=ps0[(m, t)][:],
                            lhsT=w8v[:, j, :, m, :],
                            rhs=zts[j][:].bitcast(f8)
                                .rearrange("p (n two) -> p two n", two=2)
                                [:, :, t * 512:(t + 1) * 512],
                            start=False, stop=(j == NPAIR - 1),
                            perf_mode=mybir.MatmulPerfMode.DoubleRow,
                            skip_group_check=True)

            z0 = apool.tile([128, 2 * bc], bf16, tag="actA")
            for m in range(2):
                for t in range(nt):
                    act_relu(z0[:, m * bc + t * 512: m * bc + (t + 1) * 512],
                             ps0[(m, t)][:], bias_ap("tb0", m),
                             scale=1.0 / PSCALE, on_dve=(m == 1))

            # ---- top L1-3 with residual -------------------------------
            zprev = z0
            for li, (wnm, bnm, tg) in enumerate(
                    [("tw1", "tb1", "actB"), ("tw2", "tb2", "actC"),
                     ("tw3", "tb3", "actA")]):
                zn = apool.tile([128, 2 * bc], bf16, tag=tg)
                for m in range(2):
                    for t in range(nt):
                        ps = psum(m, t)
                        for kk in range(2):
                            nc.tensor.matmul(
                                out=ps[:],
                                lhsT=mlp_w[wnm][:, kk, m * 128:(m + 1) * 128],
                                rhs=zprev[:, kk * bc + t * 512:
                                          kk * bc + (t + 1) * 512],
                                start=(kk == 0), stop=(kk == 1),
                                skip_group_check=True)
                        rl = rlpool.tile([128, 512], bf16, tag="rl", name="rl")
                        act_relu(rl[:], ps[:], bias_ap(bnm, m),
                                 on_dve=(m == 1))
                        nc.vector.tensor_add(
                            out=zn[:, m * bc + t * 512: m * bc + (t + 1) * 512],
                            in0=rl[:],
                            in1=zprev[:, m * bc + t * 512:
                                      m * bc + (t + 1) * 512])
                zprev = zn

            # ---- final layer [256 -> 1] -------------------------------
            yo = apool.tile([1, bc], f32, tag="yo")
            for t in range(nt):
                ps = psum(0, t)
                for kk in range(2):
                    nc.tensor.matmul(
                        out=ps[:1, :],
                        lhsT=mlp_w["tw4"][:, kk, 0:1],
                        rhs=zprev[:, kk * bc + t * 512:
                                  kk * bc + (t + 1) * 512],
                        start=(kk == 0), stop=(kk == 1),
                        skip_group_check=True)
                nc.scalar.activation(yo[:, t * 512:(t + 1) * 512], ps[:1, :],
                                     Ident, bias=bias_ap("tb4"))
            nc.sync.dma_start(out=out[:].rearrange("s o -> o s"), in_=yo[:])

    if split_waits:
        _split_multi_waits(nc)
    return nc


_NC_CACHE = {}


def _get_nc(bc=BC, v=V):
    key = (bc, v)
    if key not in _NC_CACHE:
        _NC_CACHE[key] = build_bass(bc=bc, v=v)
    return _NC_CACHE[key]


def make_in_maps(inputs, bc=BC, v=V, n_cores=N_CORES):
    bf = ml_dtypes.bfloat16
    x = np.ascontiguousarray(np.asarray(inputs["x"], dtype=np.float32))
    table = np.concatenate([np.asarray(inputs[f"emb{i}"]) for i in range(4)],
                           axis=0)
    assert table.shape[0] == v
    table8 = (table * np.float32(TSCALE)).astype(ml_dtypes.float8_e4m3)
    tw0 = np.asarray(inputs["tw0"], dtype=np.float32)
    tw0e = tw0[256:]                       # [26*128, 256]
    # pair layout: w8[p, j, i, h, m'] = tw0e[(2j + (p>=64))*128 + 2*(p%64)+i,
    #                                        128*h + m'] * WSCALE
    w = tw0e.reshape(NSLOT, D, 2, 128)     # [slot, d, h, m']
    w8 = np.empty((128, NPAIR, 2, 2, 128), np.float32)
    p = np.arange(128)
    for j in range(NPAIR):
        for i in range(2):
            # slot = 2j + (p>=64); d = 2*(p%64) + i
            w8[:, j, i, :, :] = w[2 * j + (p >= 64).astype(int),
                                  2 * (p % 64) + i] * np.float32(WSCALE)
    w8 = w8.reshape(128, -1).astype(ml_dtypes.float8_e4m3)

    def pack_km(wm):                       # [256, fo] -> [128, 2*fo]
        fo = wm.shape[1]
        return np.ascontiguousarray(
            wm.reshape(2, 128, fo).transpose(1, 0, 2).reshape(128, 2 * fo))

    sq = {
        "tw0h": pack_km(tw0[:256] * np.float32(PSCALE)),
        "bw1": pack_km(np.asarray(inputs["bw1"], dtype=np.float32)),
        "bw2": pack_km(np.asarray(inputs["bw2"], dtype=np.float32)),
        "tw1": pack_km(np.asarray(inputs["tw1"], dtype=np.float32)),
        "tw2": pack_km(np.asarray(inputs["tw2"], dtype=np.float32)),
        "tw3": pack_km(np.asarray(inputs["tw3"], dtype=np.float32)),
        "tw4": pack_km(np.asarray(inputs["tw4"], dtype=np.float32)),
    }
    wsq = np.concatenate([sq[nm] for nm, _ in WSQ], axis=1).astype(bf)
    assert wsq.shape == (128, WSQ_COLS)

    biasp = np.zeros((128, 15), np.float32)
    for bi, nm in enumerate(BIAS_ORDER):
        b = np.asarray(inputs[nm], dtype=np.float32).reshape(-1)
        biasp[:, 2 * bi] = b[:128]
        biasp[:, 2 * bi + 1] = b[128:]
    biasp[0, 14] = np.float32(np.asarray(inputs["tb4"]).reshape(-1)[0])

    shared = {
        "table": table8,
        "tw0e8": w8,
        "wsq": wsq,
        "bw0p": np.asarray(inputs["bw0"], dtype=np.float32).astype(bf),
        "biasp": biasp,
    }
    return [dict(shared, xs=np.ascontiguousarray(x[c * bc:(c + 1) * bc]))
            for c in range(n_cores)]


LAST_EXEC_NS = None
LAST_RES = None


def _dump_trace(res, path="/tmp/last_trace.json"):
    import json
    if not res.instructions_and_trace:
        return
    insts, tp = res.instructions_and_trace
    rows = []
    for it in insts:
        try:
            rows.append({
                "name": it.name, "op": it.op_name, "engine": str(it.engine),
                "ts": it.timestamp, "dur": it.duration,
                "end": it.end_timestamp,
            })
        except Exception:
            pass
    with open(path, "w") as f:
        json.dump({"trace_path": tp, "insts": rows}, f)


def kernel(**inputs) -> np.ndarray:
    global LAST_EXEC_NS, LAST_RES
    nc = _get_nc()
    in_maps = make_in_maps(inputs)
    trace = bool(int(os.environ.get("KERNEL_TRACE", "0")))
    res = run_bass_kernel_spmd(nc, in_maps, core_ids=list(range(N_CORES)),
                               trace=trace)
    LAST_EXEC_NS = res.exec_time_ns
    LAST_RES = res
    if trace:
        try:
            _dump_trace(res)
        except Exception as e:
            print(f"(trace dump failed: {e})")
    return np.concatenate([res.results[c]["out"] for c in range(N_CORES)],
                          axis=0)
